# revision 29
# baseline (speedup 1.0000x reference)
"""Expert-parallel MoE "behind" block + residual on 8 Trainium2 NeuronCores.

Reference computation (fp32):
    front      = inputs[:E*C].reshape(E, C, D_IN)
    expert_out = einsum("ecd,edm->ecm", front, expert_w) + expert_b
    combined   = einsum("sec,ecm->sm", combine_weights, expert_out)
    resid      = inputs[E*C:] @ residual_w + residual_b
    out        = combined * w0[:, None] + resid * w1[:, None]

Sharding (8 cores):
  Stage 1 (expert-parallel): core e computes eo_e = front_e @ W_e  [C, D_OUT],
  in two c-halves; each half is AllGathered over the cores as soon as it is
  ready (2 chunked AllGathers overlap stage-1/3 compute on the PE).
  Stage 3 (token-parallel residual): core r owns tokens S_r (512 rows) and
  accumulates (w1*resid)[S_r] @ residual_w into 8 PSUM banks.
  Stage 2 (token-parallel combine): accumulates (w0*cw)[S_r] @ eo_full into
  the same PSUM banks (w0/w1 folded into cw / resid rows on the host; exact).
  The (all-zero) bias terms are added back exactly on the host:
      out += w1 x residual_b  +  w0 * (cw.sum(c) @ expert_b)

All device matmuls contract over the SBUF partition axis, so every DRAM
operand is laid out contraction-major on the host.  The chunked AllGather
concatenates per-rank c-halves, so cwT's contraction rows are ordered
(chunk, expert, c-within-half) to match.

Modes (env TRN_KERNEL_MODE): "bf16" (default) ships bf16 operands with fp32
PSUM accumulate — measured 361 us HW exec, rel-l2 3.3e-3 vs the fp32
reference.  "fp32" is the exact fallback (plain fp32 PE at 4 cycles/row) —
1.37 ms, rel-l2 6.6e-7.  "fp32r" compiles but mis-computes on this
hardware (15% error) — do not use.
"""

import os
import numpy as np
import ml_dtypes

E, C, D_IN, D_OUT = 8, 1024, 4096, 1024
B, S = 2, 2048
TOK = B * S                 # 4096 tokens
N_CORES = 8
S_LOC = TOK // N_CORES      # 512 tokens per core
CH = C // 2                 # c-half = 512
BF16 = ml_dtypes.bfloat16

MODE = os.environ.get("TRN_KERNEL_MODE", "bf16")
LDW_OPT = os.environ.get("TRN_LDW_OPT", "0") == "1"
SKIP_LDW = os.environ.get("TRN_SKIP_LDW", "1") == "1"

_prog_cache = {}


def _patch_ldw_opt():
    """Compile this kernel's NEFF with walrus' LDWEIGHTS double-buffering
    (--enable-ldw-opt=true): hides the per-matmul 128-column weight load
    behind the previous matmul (~50 ns/MM here). Wrapped so only our
    compile is affected."""
    from concourse import bass_utils
    if getattr(bass_utils, "_ldw_opt_patched", False):
        return
    orig = bass_utils.run_command

    def patched(argv, **kw):
        argv = ["--enable-ldw-opt=true" if a == "--enable-ldw-opt=false" else a
                for a in argv]
        return orig(argv, **kw)

    bass_utils.run_command = patched
    bass_utils._ldw_opt_patched = True


def _build(mode, ldw_opt):
    import concourse.bass as bass  # noqa: F401
    import concourse.mybir as mybir
    from concourse import bacc
    from concourse.tile import TileContext, add_dep_helper

    dt = mybir.dt
    # fp32r must be the declared dtype end-to-end (the BIR verifier rejects
    # fp32-typed producers feeding fp32r matmuls), not a bitcast at the matmul
    io_dt = {"bf16": dt.bfloat16, "fp32r": dt.float32r, "fp32": dt.float32}[mode]
    mm_cast = lambda ap: ap

    nc = bacc.Bacc("TRN2", target_bir_lowering=False, debug=False, num_devices=N_CORES)

    fT = nc.declare_dram_parameter("fT", [D_IN, C], io_dt, isOutput=False)
    we = nc.declare_dram_parameter("we", [D_IN, D_OUT], io_dt, isOutput=False)
    cwT = nc.declare_dram_parameter("cwT", [E * C, S_LOC], io_dt, isOutput=False)
    riT = nc.declare_dram_parameter("riT", [D_IN, S_LOC], io_dt, isOutput=False)
    rw = nc.declare_dram_parameter("rw", [D_IN, D_OUT], io_dt, isOutput=False)
    out = nc.declare_dram_parameter("out", [S_LOC, D_OUT], dt.float32, isOutput=True)

    # variant tag in a tensor name so differently-compiled builds never share
    # a jax compile-cache entry
    nc.dram_tensor(f"variant_{mode}_{int(ldw_opt)}_{int(SKIP_LDW)}", [1, 1], dt.float32)

    ag_in = [nc.dram_tensor(f"ag_in{h}", [CH, D_OUT], io_dt) for h in range(2)]
    ag_out = [nc.dram_tensor(f"ag_out{h}", [N_CORES * CH, D_OUT], io_dt,
                             addr_space="Shared") for h in range(2)]

    KT = D_IN // 128            # 32 contraction tiles
    SUB = 4                     # k-subtiles per DMA'd block
    NBLK = KT // SUB            # 8 blocks
    ECT = (E * C) // 128        # 64 combine contraction tiles
    NFREE = 512                 # ISA cap: s3d3_mm_num_elements <= 512
    NJ = D_OUT // NFREE
    rearr = lambda a: a.rearrange("(n p) d -> p n d", p=128)

    S1SUB = 2                   # finer stage-1 blocks: deeper prefetch pipeline
    S1BLK = KT // S1SUB         # 16 blocks

    bf16_mode = io_dt == dt.bfloat16
    B_FT, B_WE, B_RI, B_RW, B_CW, B_EOAG, B_EO = (
        (8, 8, 3, 3, 3, 6, 2) if bf16_mode else (4, 4, 2, 2, 2, 2, 1))
    with TileContext(nc) as tc:
        with tc.tile_pool(name="p_ft", bufs=B_FT) as p_ft, \
             tc.tile_pool(name="p_we", bufs=B_WE) as p_we, \
             tc.tile_pool(name="p_ri", bufs=B_RI) as p_ri, \
             tc.tile_pool(name="p_rw", bufs=B_RW) as p_rw, \
             tc.tile_pool(name="p_cw", bufs=B_CW) as p_cw, \
             tc.tile_pool(name="p_eoag", bufs=B_EOAG) as p_eoag, \
             tc.tile_pool(name="p_eo", bufs=B_EO) as p_eo, \
             tc.tile_pool(name="p_out", bufs=1) as p_out, \
             tc.tile_pool(name="psum", bufs=1, space="PSUM") as p_ps:


            def mm_pair(psrow, lhsT_ap, rhs_of_j, start, stop):
                """Two matmuls sharing one stationary operand: the second
                skips its LDWEIGHTS (identical weights already in the array)
                and is order-pinned right after the first."""
                prev = None
                for j in range(NJ):
                    m = nc.tensor.matmul(psrow[j], lhsT_ap, rhs_of_j(j),
                                         start=start, stop=stop)
                    # fp32's two-pass matmul requires self-loading weights
                    if j > 0 and SKIP_LDW and bf16_mode:
                        m.ins.ldweights = False
                        add_dep_helper(m.ins, prev.ins, False, "weight-reuse pair order")
                    prev = m

            def psum_tiles(tagp):
                return [[p_ps.tile([128, NFREE], dt.float32,
                                   name=f"{tagp}_{i}_{j}", tag=f"ps_{i}_{j}")
                         for j in range(NJ)] for i in range(4)]

            # ------------- Stage 1: eo_e = fT.T @ we, by c-halves ------------
            last_we_dma = None
            for ch in range(2):
                c0 = ch * CH
                psums = psum_tiles(f"s1h{ch}")
                for blk in range(S1BLK):
                    r0 = blk * 128 * S1SUB
                    # only this c-half's columns of fT are needed
                    ft_t = p_ft.tile([128, S1SUB, CH], io_dt, tag="ft", name=f"ft_{ch}_{blk}")
                    we_t = p_we.tile([128, S1SUB, D_OUT], io_dt, tag="we", name=f"we_{ch}_{blk}")
                    # balance the two HWDGE rings: we is swept twice (8 MB per
                    # c-half) while ft is halved, so ship ch1's we re-read on
                    # the sync ring
                    we_eng = nc.scalar if ch == 0 else nc.sync
                    if ch == 0 and blk == 0:
                        # per-sub/j loads: the first matmul starts after 256 KiB,
                        # not after the whole block
                        for sub in range(S1SUB):
                            nc.sync.dma_start(
                                out=ft_t[:, sub:sub + 1, :],
                                in_=rearr(fT[r0 + sub * 128:r0 + (sub + 1) * 128, c0:c0 + CH]))
                            for j in range(NJ):
                                last_we_dma = nc.scalar.dma_start(
                                    out=we_t[:, sub:sub + 1, j * NFREE:(j + 1) * NFREE],
                                    in_=rearr(we[r0 + sub * 128:r0 + (sub + 1) * 128,
                                                 j * NFREE:(j + 1) * NFREE]))
                    else:
                        nc.sync.dma_start(
                            out=ft_t, in_=rearr(fT[r0:r0 + 128 * S1SUB, c0:c0 + CH]))
                        last_we_dma = we_eng.dma_start(
                            out=we_t, in_=rearr(we[r0:r0 + 128 * S1SUB, :]))
                    for sub in range(S1SUB):
                        kt = blk * S1SUB + sub
                        for i in range(4):
                            mm_pair(psums[i],
                                    mm_cast(ft_t[:, sub, i * 128:(i + 1) * 128]),
                                    lambda j, sub=sub: mm_cast(we_t[:, sub, j * NFREE:(j + 1) * NFREE]),
                                    start=(kt == 0), stop=(kt == KT - 1))
                eo_half = p_eo.tile([128, 4, D_OUT], io_dt, tag="eo", name=f"eo_{ch}")
                for i in range(4):
                    for j in range(NJ):
                        nc.vector.tensor_copy(out=eo_half[:, i, j * NFREE:(j + 1) * NFREE],
                                              in_=psums[i][j])
                # gpsimd (SWDGE) queue: keeps this late-gated write out of the
                # HWDGE FIFOs so it can't head-of-line block operand loads
                nc.gpsimd.dma_start(out=rearr(ag_in[ch][:]), in_=eo_half)
                # chunked AllGather: starts while the PE grinds the next phase
                nc.gpsimd.collective_compute(
                    "AllGather", mybir.AluOpType.bypass,
                    replica_groups=[list(range(N_CORES))],
                    ins=[ag_in[ch][:].opt()], outs=[ag_out[ch][:].opt()])

            # ------------- Stage 3: resid partial (w1 folded) ----------------
            psums = psum_tiles("s23")
            last_rw_dma = None
            for blk in range(NBLK):
                ri_t = p_ri.tile([128, SUB, S_LOC], io_dt, tag="ri", name=f"ri_{blk}")
                nc.sync.dma_start(out=ri_t, in_=rearr(riT[blk * 512:(blk + 1) * 512, :]))
                rw_t = p_rw.tile([128, SUB, D_OUT], io_dt, tag="rw", name=f"rw_{blk}")
                last_rw_dma = nc.scalar.dma_start(
                    out=rw_t, in_=rearr(rw[blk * 512:(blk + 1) * 512, :]))
                for sub in range(SUB):
                    kt = blk * SUB + sub
                    for i in range(4):
                        mm_pair(psums[i],
                                mm_cast(ri_t[:, sub, i * 128:(i + 1) * 128]),
                                lambda j, sub=sub: mm_cast(rw_t[:, sub, j * NFREE:(j + 1) * NFREE]),
                                start=(kt == 0), stop=False)

            # ------------- Stage 2: combine partial (w0 folded) --------------
            out_sb = p_out.tile([128, 4, D_OUT], dt.float32)
            prev_eoag_dma = None
            for blk in range(ECT // SUB):
                half = blk // 8              # ag chunk this block reads
                r0 = (blk % 8) * 512
                cw_t = p_cw.tile([128, SUB, S_LOC], io_dt, tag="cw", name=f"cw_{blk}")
                nc.sync.dma_start(out=cw_t, in_=rearr(cwT[blk * 512:(blk + 1) * 512, :]))
                eo_t = p_eoag.tile([128, SUB, D_OUT], io_dt, tag="eoag", name=f"eoag_{blk}")
                eoag_dma = nc.scalar.dma_start(out=eo_t, in_=rearr(ag_out[half][r0:r0 + 512, :]))
                # Scalar-queue order: eoag loads wait on the AllGathers, so pin
                # them after every stage-1/3 operand load and in block order —
                # otherwise the scheduler can hoist one ahead and head-of-line
                # block the HWDGE FIFO on the collective (measured 44 us stall).
                prev = prev_eoag_dma if prev_eoag_dma is not None else (
                    last_rw_dma if last_rw_dma is not None else last_we_dma)
                if prev is not None:
                    add_dep_helper(eoag_dma.ins, prev.ins, False,
                                   "eoag after stage-1/3 loads, in block order")
                prev_eoag_dma = eoag_dma
                last_blk = blk == ECT // SUB - 1
                if not last_blk:
                    for sub in range(SUB):
                        for i in range(4):
                            mm_pair(psums[i],
                                    mm_cast(cw_t[:, sub, i * 128:(i + 1) * 128]),
                                    lambda j, sub=sub: mm_cast(eo_t[:, sub, j * NFREE:(j + 1) * NFREE]),
                                    start=False, stop=False)
                else:
                    # last block: finish groups one at a time so the PSUM->SBUF
                    # copies and output DMAs overlap the remaining matmuls
                    for i in range(4):
                        for j in range(NJ):
                            for sub in range(SUB):
                                nc.tensor.matmul(
                                    psums[i][j],
                                    mm_cast(cw_t[:, sub, i * 128:(i + 1) * 128]),
                                    mm_cast(eo_t[:, sub, j * NFREE:(j + 1) * NFREE]),
                                    start=False, stop=(sub == SUB - 1))
                            nc.vector.tensor_copy(
                                out=out_sb[:, i, j * NFREE:(j + 1) * NFREE],
                                in_=psums[i][j])
                        nc.sync.dma_start(
                            out=out[i * 128:(i + 1) * 128, :].rearrange("(n p) d -> p n d", p=128),
                            in_=out_sb[:, i:i + 1, :])

    nc.finalize()
    return nc


def _get_prog(mode, ldw_opt):
    key = (mode, ldw_opt)
    if key not in _prog_cache:
        if ldw_opt:
            _patch_ldw_opt()
        _prog_cache[key] = _build(mode, ldw_opt)
    return _prog_cache[key]


def _prep_in_maps(inputs, expert_w, residual_w, combine_weights, residual_weight, mode):
    np_dt = BF16 if mode == "bf16" else np.float32
    front = inputs[:E * C].reshape(E, C, D_IN)
    resid = inputs[E * C:]                       # [TOK, D_IN]
    rwt = residual_weight.reshape(TOK, 2)
    w0, w1 = rwt[:, 0], rwt[:, 1]

    rw_cast = np.ascontiguousarray(residual_w.astype(np_dt))
    resid_s = resid * w1[:, None]                # fold w1 (fp32)
    in_maps = []
    for r in range(N_CORES):
        sl = slice(r * S_LOC, (r + 1) * S_LOC)
        fT = np.ascontiguousarray(front[r].T.astype(np_dt))              # [D_IN, C]
        we = np.ascontiguousarray(expert_w[r].astype(np_dt))             # [D_IN, D_OUT]
        cw_s = combine_weights[sl] * w0[sl, None, None]                  # [S_LOC, E, C]
        # contraction rows ordered (c-half chunk, expert, c-within-half) to
        # match the chunked AllGather's concatenation
        cwT = np.ascontiguousarray(
            cw_s.reshape(S_LOC, E, 2, CH).transpose(2, 1, 3, 0).reshape(E * C, S_LOC)
            .astype(np_dt))
        riT = np.ascontiguousarray(resid_s[sl].T.astype(np_dt))          # [D_IN, S_LOC]
        in_maps.append({"fT": fT, "we": we, "cwT": cwT, "riT": riT, "rw": rw_cast})
    return in_maps


def _run(inputs, expert_w, expert_b, residual_w, residual_b,
         combine_weights, residual_weight, mode=None, ldw_opt=None, trace=False):
    import jax
    try:
        if jax.config.jax_compilation_cache_dir is None:
            jax.config.update("jax_compilation_cache_dir", "/tmp/jax_cache_trn_moe")
            jax.config.update("jax_persistent_cache_min_compile_time_secs", 0.5)
    except Exception:
        pass
    from concourse.bass_utils import run_bass_kernel_spmd

    mode = mode or MODE
    ldw_opt = LDW_OPT if ldw_opt is None else ldw_opt
    inputs = np.asarray(inputs, dtype=np.float32)
    expert_w = np.asarray(expert_w, dtype=np.float32)
    expert_b = np.asarray(expert_b, dtype=np.float32)
    residual_w = np.asarray(residual_w, dtype=np.float32)
    residual_b = np.asarray(residual_b, dtype=np.float32)
    combine_weights = np.asarray(combine_weights, dtype=np.float32)
    residual_weight = np.asarray(residual_weight, dtype=np.float32)

    nc = _get_prog(mode, ldw_opt)
    in_maps = _prep_in_maps(inputs, expert_w, residual_w, combine_weights,
                            residual_weight, mode)
    res = run_bass_kernel_spmd(nc, in_maps, list(range(N_CORES)), trace=trace)
    out = np.concatenate([res.results[r]["out"] for r in range(N_CORES)], axis=0)

    # exact bias contributions (zero in practice, but keep the math honest)
    rwt = residual_weight.reshape(TOK, 2)
    if residual_b.any():
        out = out + rwt[:, 1:2] * residual_b[None, :]
    if expert_b.any():
        cs = combine_weights.sum(axis=2)                    # [TOK, E]
        out = out + rwt[:, 0:1] * (cs @ expert_b)
    return out.reshape(B, S, D_OUT).astype(np.float32), res


def kernel(**kw):
    out, _ = _run(**kw)
    return out


# revision 30
# speedup vs baseline: 1.0180x; 1.0180x over previous
"""Expert-parallel MoE "behind" block + residual on 8 Trainium2 NeuronCores.

Reference computation (fp32):
    front      = inputs[:E*C].reshape(E, C, D_IN)
    expert_out = einsum("ecd,edm->ecm", front, expert_w) + expert_b
    combined   = einsum("sec,ecm->sm", combine_weights, expert_out)
    resid      = inputs[E*C:] @ residual_w + residual_b
    out        = combined * w0[:, None] + resid * w1[:, None]

Sharding (8 cores):
  Stage 1 (expert-parallel): core e computes eo_e = front_e @ W_e  [C, D_OUT],
  in two c-halves; each half is AllGathered over the cores as soon as it is
  ready (2 chunked AllGathers overlap stage-1/3 compute on the PE).
  Stage 3 (token-parallel residual): core r owns tokens S_r (512 rows) and
  accumulates (w1*resid)[S_r] @ residual_w into 8 PSUM banks.
  Stage 2 (token-parallel combine): accumulates (w0*cw)[S_r] @ eo_full into
  the same PSUM banks (w0/w1 folded into cw / resid rows on the host; exact).
  The (all-zero) bias terms are added back exactly on the host:
      out += w1 x residual_b  +  w0 * (cw.sum(c) @ expert_b)

All device matmuls contract over the SBUF partition axis, so every DRAM
operand is laid out contraction-major on the host.  The chunked AllGather
concatenates per-rank c-halves, so cwT's contraction rows are ordered
(chunk, expert, c-within-half) to match.

Modes (env TRN_KERNEL_MODE): "bf16" (default) ships bf16 operands with fp32
PSUM accumulate — measured 361 us HW exec, rel-l2 3.3e-3 vs the fp32
reference.  "fp32" is the exact fallback (plain fp32 PE at 4 cycles/row) —
1.37 ms, rel-l2 6.6e-7.  "fp32r" compiles but mis-computes on this
hardware (15% error) — do not use.
"""

import os
import numpy as np
import ml_dtypes

E, C, D_IN, D_OUT = 8, 1024, 4096, 1024
B, S = 2, 2048
TOK = B * S                 # 4096 tokens
N_CORES = 8
S_LOC = TOK // N_CORES      # 512 tokens per core
CH = C // 2                 # c-half = 512
BF16 = ml_dtypes.bfloat16

MODE = os.environ.get("TRN_KERNEL_MODE", "bf16")
LDW_OPT = os.environ.get("TRN_LDW_OPT", "0") == "1"
SKIP_LDW = os.environ.get("TRN_SKIP_LDW", "1") == "1"

_prog_cache = {}


def _patch_ldw_opt():
    """Compile this kernel's NEFF with walrus' LDWEIGHTS double-buffering
    (--enable-ldw-opt=true): hides the per-matmul 128-column weight load
    behind the previous matmul (~50 ns/MM here). Wrapped so only our
    compile is affected."""
    from concourse import bass_utils
    if getattr(bass_utils, "_ldw_opt_patched", False):
        return
    orig = bass_utils.run_command

    def patched(argv, **kw):
        argv = ["--enable-ldw-opt=true" if a == "--enable-ldw-opt=false" else a
                for a in argv]
        return orig(argv, **kw)

    bass_utils.run_command = patched
    bass_utils._ldw_opt_patched = True


def _build(mode, ldw_opt):
    import concourse.bass as bass  # noqa: F401
    import concourse.mybir as mybir
    from concourse import bacc
    from concourse.tile import TileContext, add_dep_helper

    dt = mybir.dt
    # fp32r must be the declared dtype end-to-end (the BIR verifier rejects
    # fp32-typed producers feeding fp32r matmuls), not a bitcast at the matmul
    io_dt = {"bf16": dt.bfloat16, "fp32r": dt.float32r, "fp32": dt.float32}[mode]
    mm_cast = lambda ap: ap

    nc = bacc.Bacc("TRN2", target_bir_lowering=False, debug=False, num_devices=N_CORES)

    fT = nc.declare_dram_parameter("fT", [D_IN, C], io_dt, isOutput=False)
    we = nc.declare_dram_parameter("we", [D_IN, D_OUT], io_dt, isOutput=False)
    cwT = nc.declare_dram_parameter("cwT", [E * C, S_LOC], io_dt, isOutput=False)
    riT = nc.declare_dram_parameter("riT", [D_IN, S_LOC], io_dt, isOutput=False)
    rw = nc.declare_dram_parameter("rw", [D_IN, D_OUT], io_dt, isOutput=False)
    out = nc.declare_dram_parameter("out", [S_LOC, D_OUT], dt.float32, isOutput=True)

    # variant tag in a tensor name so differently-compiled builds never share
    # a jax compile-cache entry
    nc.dram_tensor(f"variant_{mode}_{int(ldw_opt)}_{int(SKIP_LDW)}", [1, 1], dt.float32)

    ag_in = [nc.dram_tensor(f"ag_in{h}", [CH, D_OUT], io_dt) for h in range(2)]
    ag_out = [nc.dram_tensor(f"ag_out{h}", [N_CORES * CH, D_OUT], io_dt,
                             addr_space="Shared") for h in range(2)]

    KT = D_IN // 128            # 32 contraction tiles
    SUB = 4                     # k-subtiles per DMA'd block
    NBLK = KT // SUB            # 8 blocks
    ECT = (E * C) // 128        # 64 combine contraction tiles
    NFREE = 512                 # ISA cap: s3d3_mm_num_elements <= 512
    NJ = D_OUT // NFREE
    rearr = lambda a: a.rearrange("(n p) d -> p n d", p=128)

    S1SUB = 2                   # finer stage-1 blocks: deeper prefetch pipeline
    S1BLK = KT // S1SUB         # 16 blocks

    bf16_mode = io_dt == dt.bfloat16
    B_FT, B_WE, B_RI, B_RW, B_CW, B_EOAG, B_EO = (
        (8, 8, 3, 3, 3, 6, 2) if bf16_mode else (4, 4, 2, 2, 2, 2, 1))
    with TileContext(nc) as tc:
        with tc.tile_pool(name="p_ft", bufs=B_FT) as p_ft, \
             tc.tile_pool(name="p_we", bufs=B_WE) as p_we, \
             tc.tile_pool(name="p_ri", bufs=B_RI) as p_ri, \
             tc.tile_pool(name="p_rw", bufs=B_RW) as p_rw, \
             tc.tile_pool(name="p_cw", bufs=B_CW) as p_cw, \
             tc.tile_pool(name="p_eoag", bufs=B_EOAG) as p_eoag, \
             tc.tile_pool(name="p_eo", bufs=B_EO) as p_eo, \
             tc.tile_pool(name="p_out", bufs=1) as p_out, \
             tc.tile_pool(name="psum", bufs=1, space="PSUM") as p_ps:


            def mm_pair(psrow, lhsT_ap, rhs_of_j, start, stop):
                """Two matmuls sharing one stationary operand: the second
                skips its LDWEIGHTS (identical weights already in the array)
                and is order-pinned right after the first."""
                prev = None
                for j in range(NJ):
                    m = nc.tensor.matmul(psrow[j], lhsT_ap, rhs_of_j(j),
                                         start=start, stop=stop)
                    # fp32's two-pass matmul requires self-loading weights
                    if j > 0 and SKIP_LDW and bf16_mode:
                        m.ins.ldweights = False
                        add_dep_helper(m.ins, prev.ins, False, "weight-reuse pair order")
                    prev = m

            def psum_tiles(tagp):
                return [[p_ps.tile([128, NFREE], dt.float32,
                                   name=f"{tagp}_{i}_{j}", tag=f"ps_{i}_{j}")
                         for j in range(NJ)] for i in range(4)]

            # ------------- Stage 1: eo_e = fT.T @ we, by c-halves ------------
            last_we_dma = None
            for ch in range(2):
                c0 = ch * CH
                psums = psum_tiles(f"s1h{ch}")
                for blk in range(S1BLK):
                    r0 = blk * 128 * S1SUB
                    # only this c-half's columns of fT are needed
                    ft_t = p_ft.tile([128, S1SUB, CH], io_dt, tag="ft", name=f"ft_{ch}_{blk}")
                    we_t = p_we.tile([128, S1SUB, D_OUT], io_dt, tag="we", name=f"we_{ch}_{blk}")
                    if ch == 0 and blk == 0:
                        # per-sub loads: the first matmul starts after 256 KiB,
                        # not after the whole block
                        for sub in range(S1SUB):
                            nc.sync.dma_start(
                                out=ft_t[:, sub:sub + 1, :],
                                in_=rearr(fT[r0 + sub * 128:r0 + (sub + 1) * 128, c0:c0 + CH]))
                            last_we_dma = nc.scalar.dma_start(
                                out=we_t[:, sub:sub + 1, :],
                                in_=rearr(we[r0 + sub * 128:r0 + (sub + 1) * 128, :]))
                    else:
                        nc.sync.dma_start(
                            out=ft_t, in_=rearr(fT[r0:r0 + 128 * S1SUB, c0:c0 + CH]))
                        # scalar queue: second HWDGE ring, parallel with sync's
                        last_we_dma = nc.scalar.dma_start(
                            out=we_t, in_=rearr(we[r0:r0 + 128 * S1SUB, :]))
                    for sub in range(S1SUB):
                        kt = blk * S1SUB + sub
                        for i in range(4):
                            mm_pair(psums[i],
                                    mm_cast(ft_t[:, sub, i * 128:(i + 1) * 128]),
                                    lambda j, sub=sub: mm_cast(we_t[:, sub, j * NFREE:(j + 1) * NFREE]),
                                    start=(kt == 0), stop=(kt == KT - 1))
                eo_half = p_eo.tile([128, 4, D_OUT], io_dt, tag="eo", name=f"eo_{ch}")
                for i in range(4):
                    for j in range(NJ):
                        nc.vector.tensor_copy(out=eo_half[:, i, j * NFREE:(j + 1) * NFREE],
                                              in_=psums[i][j])
                # gpsimd (SWDGE) queue: keeps this late-gated write out of the
                # HWDGE FIFOs so it can't head-of-line block operand loads
                nc.gpsimd.dma_start(out=rearr(ag_in[ch][:]), in_=eo_half)
                # chunked AllGather: starts while the PE grinds the next phase
                nc.gpsimd.collective_compute(
                    "AllGather", mybir.AluOpType.bypass,
                    replica_groups=[list(range(N_CORES))],
                    ins=[ag_in[ch][:].opt()], outs=[ag_out[ch][:].opt()])

            # ------------- Stage 3: resid partial (w1 folded) ----------------
            psums = psum_tiles("s23")
            last_rw_dma = None
            for blk in range(NBLK):
                ri_t = p_ri.tile([128, SUB, S_LOC], io_dt, tag="ri", name=f"ri_{blk}")
                nc.sync.dma_start(out=ri_t, in_=rearr(riT[blk * 512:(blk + 1) * 512, :]))
                rw_t = p_rw.tile([128, SUB, D_OUT], io_dt, tag="rw", name=f"rw_{blk}")
                last_rw_dma = nc.scalar.dma_start(
                    out=rw_t, in_=rearr(rw[blk * 512:(blk + 1) * 512, :]))
                for sub in range(SUB):
                    kt = blk * SUB + sub
                    for i in range(4):
                        mm_pair(psums[i],
                                mm_cast(ri_t[:, sub, i * 128:(i + 1) * 128]),
                                lambda j, sub=sub: mm_cast(rw_t[:, sub, j * NFREE:(j + 1) * NFREE]),
                                start=(kt == 0), stop=False)

            # ------------- Stage 2: combine partial (w0 folded) --------------
            out_sb = p_out.tile([128, 4, D_OUT], dt.float32)
            prev_eoag_dma = None
            for blk in range(ECT // SUB):
                half = blk // 8              # ag chunk this block reads
                r0 = (blk % 8) * 512
                cw_t = p_cw.tile([128, SUB, S_LOC], io_dt, tag="cw", name=f"cw_{blk}")
                nc.sync.dma_start(out=cw_t, in_=rearr(cwT[blk * 512:(blk + 1) * 512, :]))
                eo_t = p_eoag.tile([128, SUB, D_OUT], io_dt, tag="eoag", name=f"eoag_{blk}")
                eoag_dma = nc.scalar.dma_start(out=eo_t, in_=rearr(ag_out[half][r0:r0 + 512, :]))
                # Scalar-queue order: eoag loads wait on the AllGathers, so pin
                # them after every stage-1/3 operand load and in block order —
                # otherwise the scheduler can hoist one ahead and head-of-line
                # block the HWDGE FIFO on the collective (measured 44 us stall).
                prev = prev_eoag_dma if prev_eoag_dma is not None else (
                    last_rw_dma if last_rw_dma is not None else last_we_dma)
                if prev is not None:
                    add_dep_helper(eoag_dma.ins, prev.ins, False,
                                   "eoag after stage-1/3 loads, in block order")
                prev_eoag_dma = eoag_dma
                last_blk = blk == ECT // SUB - 1
                if not last_blk:
                    for sub in range(SUB):
                        for i in range(4):
                            mm_pair(psums[i],
                                    mm_cast(cw_t[:, sub, i * 128:(i + 1) * 128]),
                                    lambda j, sub=sub: mm_cast(eo_t[:, sub, j * NFREE:(j + 1) * NFREE]),
                                    start=False, stop=False)
                else:
                    # last block: finish groups one at a time so the PSUM->SBUF
                    # copies and output DMAs overlap the remaining matmuls
                    for i in range(4):
                        for j in range(NJ):
                            for sub in range(SUB):
                                nc.tensor.matmul(
                                    psums[i][j],
                                    mm_cast(cw_t[:, sub, i * 128:(i + 1) * 128]),
                                    mm_cast(eo_t[:, sub, j * NFREE:(j + 1) * NFREE]),
                                    start=False, stop=(sub == SUB - 1))
                            nc.vector.tensor_copy(
                                out=out_sb[:, i, j * NFREE:(j + 1) * NFREE],
                                in_=psums[i][j])
                        nc.sync.dma_start(
                            out=out[i * 128:(i + 1) * 128, :].rearrange("(n p) d -> p n d", p=128),
                            in_=out_sb[:, i:i + 1, :])

    nc.finalize()
    return nc


def _get_prog(mode, ldw_opt):
    key = (mode, ldw_opt)
    if key not in _prog_cache:
        if ldw_opt:
            _patch_ldw_opt()
        _prog_cache[key] = _build(mode, ldw_opt)
    return _prog_cache[key]


def _prep_in_maps(inputs, expert_w, residual_w, combine_weights, residual_weight, mode):
    np_dt = BF16 if mode == "bf16" else np.float32
    front = inputs[:E * C].reshape(E, C, D_IN)
    resid = inputs[E * C:]                       # [TOK, D_IN]
    rwt = residual_weight.reshape(TOK, 2)
    w0, w1 = rwt[:, 0], rwt[:, 1]

    rw_cast = np.ascontiguousarray(residual_w.astype(np_dt))
    resid_s = resid * w1[:, None]                # fold w1 (fp32)
    in_maps = []
    for r in range(N_CORES):
        sl = slice(r * S_LOC, (r + 1) * S_LOC)
        fT = np.ascontiguousarray(front[r].T.astype(np_dt))              # [D_IN, C]
        we = np.ascontiguousarray(expert_w[r].astype(np_dt))             # [D_IN, D_OUT]
        cw_s = combine_weights[sl] * w0[sl, None, None]                  # [S_LOC, E, C]
        # contraction rows ordered (c-half chunk, expert, c-within-half) to
        # match the chunked AllGather's concatenation
        cwT = np.ascontiguousarray(
            cw_s.reshape(S_LOC, E, 2, CH).transpose(2, 1, 3, 0).reshape(E * C, S_LOC)
            .astype(np_dt))
        riT = np.ascontiguousarray(resid_s[sl].T.astype(np_dt))          # [D_IN, S_LOC]
        in_maps.append({"fT": fT, "we": we, "cwT": cwT, "riT": riT, "rw": rw_cast})
    return in_maps


def _run(inputs, expert_w, expert_b, residual_w, residual_b,
         combine_weights, residual_weight, mode=None, ldw_opt=None, trace=False):
    import jax
    try:
        if jax.config.jax_compilation_cache_dir is None:
            jax.config.update("jax_compilation_cache_dir", "/tmp/jax_cache_trn_moe")
            jax.config.update("jax_persistent_cache_min_compile_time_secs", 0.5)
    except Exception:
        pass
    from concourse.bass_utils import run_bass_kernel_spmd

    mode = mode or MODE
    ldw_opt = LDW_OPT if ldw_opt is None else ldw_opt
    inputs = np.asarray(inputs, dtype=np.float32)
    expert_w = np.asarray(expert_w, dtype=np.float32)
    expert_b = np.asarray(expert_b, dtype=np.float32)
    residual_w = np.asarray(residual_w, dtype=np.float32)
    residual_b = np.asarray(residual_b, dtype=np.float32)
    combine_weights = np.asarray(combine_weights, dtype=np.float32)
    residual_weight = np.asarray(residual_weight, dtype=np.float32)

    nc = _get_prog(mode, ldw_opt)
    in_maps = _prep_in_maps(inputs, expert_w, residual_w, combine_weights,
                            residual_weight, mode)
    res = run_bass_kernel_spmd(nc, in_maps, list(range(N_CORES)), trace=trace)
    out = np.concatenate([res.results[r]["out"] for r in range(N_CORES)], axis=0)

    # exact bias contributions (zero in practice, but keep the math honest)
    rwt = residual_weight.reshape(TOK, 2)
    if residual_b.any():
        out = out + rwt[:, 1:2] * residual_b[None, :]
    if expert_b.any():
        cs = combine_weights.sum(axis=2)                    # [TOK, E]
        out = out + rwt[:, 0:1] * (cs @ expert_b)
    return out.reshape(B, S, D_OUT).astype(np.float32), res


def kernel(**kw):
    out, _ = _run(**kw)
    return out


# revision 31
# speedup vs baseline: 1.0199x; 1.0019x over previous
"""Expert-parallel MoE "behind" block + residual on 8 Trainium2 NeuronCores.

Reference computation (fp32):
    front      = inputs[:E*C].reshape(E, C, D_IN)
    expert_out = einsum("ecd,edm->ecm", front, expert_w) + expert_b
    combined   = einsum("sec,ecm->sm", combine_weights, expert_out)
    resid      = inputs[E*C:] @ residual_w + residual_b
    out        = combined * w0[:, None] + resid * w1[:, None]

Sharding (8 cores):
  Stage 1 (expert-parallel): core e computes eo_e = front_e @ W_e  [C, D_OUT],
  in two c-halves; each half is AllGathered over the cores as soon as it is
  ready (2 chunked AllGathers overlap stage-1/3 compute on the PE).
  Stage 3 (token-parallel residual): core r owns tokens S_r (512 rows) and
  accumulates (w1*resid)[S_r] @ residual_w into 8 PSUM banks.
  Stage 2 (token-parallel combine): accumulates (w0*cw)[S_r] @ eo_full into
  the same PSUM banks (w0/w1 folded into cw / resid rows on the host; exact).
  The (all-zero) bias terms are added back exactly on the host:
      out += w1 x residual_b  +  w0 * (cw.sum(c) @ expert_b)

All device matmuls contract over the SBUF partition axis, so every DRAM
operand is laid out contraction-major on the host.  The chunked AllGather
concatenates per-rank c-halves, so cwT's contraction rows are ordered
(chunk, expert, c-within-half) to match.

Modes (env TRN_KERNEL_MODE): "bf16" (default) ships bf16 operands with fp32
PSUM accumulate — measured 361 us HW exec, rel-l2 3.3e-3 vs the fp32
reference.  "fp32" is the exact fallback (plain fp32 PE at 4 cycles/row) —
1.37 ms, rel-l2 6.6e-7.  "fp32r" compiles but mis-computes on this
hardware (15% error) — do not use.
"""

import os
import numpy as np
import ml_dtypes

E, C, D_IN, D_OUT = 8, 1024, 4096, 1024
B, S = 2, 2048
TOK = B * S                 # 4096 tokens
N_CORES = 8
S_LOC = TOK // N_CORES      # 512 tokens per core
CH = C // 2                 # c-half = 512
BF16 = ml_dtypes.bfloat16

MODE = os.environ.get("TRN_KERNEL_MODE", "bf16")
LDW_OPT = os.environ.get("TRN_LDW_OPT", "0") == "1"
SKIP_LDW = os.environ.get("TRN_SKIP_LDW", "1") == "1"

_prog_cache = {}


def _patch_ldw_opt():
    """Compile this kernel's NEFF with walrus' LDWEIGHTS double-buffering
    (--enable-ldw-opt=true): hides the per-matmul 128-column weight load
    behind the previous matmul (~50 ns/MM here). Wrapped so only our
    compile is affected."""
    from concourse import bass_utils
    if getattr(bass_utils, "_ldw_opt_patched", False):
        return
    orig = bass_utils.run_command

    def patched(argv, **kw):
        argv = ["--enable-ldw-opt=true" if a == "--enable-ldw-opt=false" else a
                for a in argv]
        return orig(argv, **kw)

    bass_utils.run_command = patched
    bass_utils._ldw_opt_patched = True


def _build(mode, ldw_opt):
    import concourse.bass as bass  # noqa: F401
    import concourse.mybir as mybir
    from concourse import bacc
    from concourse.tile import TileContext, add_dep_helper

    dt = mybir.dt
    # fp32r must be the declared dtype end-to-end (the BIR verifier rejects
    # fp32-typed producers feeding fp32r matmuls), not a bitcast at the matmul
    io_dt = {"bf16": dt.bfloat16, "fp32r": dt.float32r, "fp32": dt.float32}[mode]
    mm_cast = lambda ap: ap

    nc = bacc.Bacc("TRN2", target_bir_lowering=False, debug=False, num_devices=N_CORES)

    fT = nc.declare_dram_parameter("fT", [D_IN, C], io_dt, isOutput=False)
    we = nc.declare_dram_parameter("we", [D_IN, D_OUT], io_dt, isOutput=False)
    cwT = nc.declare_dram_parameter("cwT", [E * C, S_LOC], io_dt, isOutput=False)
    riT = nc.declare_dram_parameter("riT", [D_IN, S_LOC], io_dt, isOutput=False)
    rw = nc.declare_dram_parameter("rw", [D_IN, D_OUT], io_dt, isOutput=False)
    out = nc.declare_dram_parameter("out", [S_LOC, D_OUT], dt.float32, isOutput=True)

    # variant tag in a tensor name so differently-compiled builds never share
    # a jax compile-cache entry
    nc.dram_tensor(f"variant_{mode}_{int(ldw_opt)}_{int(SKIP_LDW)}", [1, 1], dt.float32)

    ag_in = [nc.dram_tensor(f"ag_in{h}", [CH, D_OUT], io_dt) for h in range(2)]
    ag_out = [nc.dram_tensor(f"ag_out{h}", [N_CORES * CH, D_OUT], io_dt,
                             addr_space="Shared") for h in range(2)]

    KT = D_IN // 128            # 32 contraction tiles
    SUB = 4                     # k-subtiles per DMA'd block
    NBLK = KT // SUB            # 8 blocks
    ECT = (E * C) // 128        # 64 combine contraction tiles
    NFREE = 512                 # ISA cap: s3d3_mm_num_elements <= 512
    NJ = D_OUT // NFREE
    rearr = lambda a: a.rearrange("(n p) d -> p n d", p=128)

    S1SUB = 2                   # finer stage-1 blocks: deeper prefetch pipeline
    S1BLK = KT // S1SUB         # 16 blocks

    bf16_mode = io_dt == dt.bfloat16
    B_FT, B_WE, B_RI, B_RW, B_CW, B_EOAG, B_EO = (
        (12, 9, 3, 3, 3, 6, 2) if bf16_mode else (4, 4, 2, 2, 2, 2, 1))
    with TileContext(nc) as tc:
        with tc.tile_pool(name="p_ft", bufs=B_FT) as p_ft, \
             tc.tile_pool(name="p_we", bufs=B_WE) as p_we, \
             tc.tile_pool(name="p_ri", bufs=B_RI) as p_ri, \
             tc.tile_pool(name="p_rw", bufs=B_RW) as p_rw, \
             tc.tile_pool(name="p_cw", bufs=B_CW) as p_cw, \
             tc.tile_pool(name="p_eoag", bufs=B_EOAG) as p_eoag, \
             tc.tile_pool(name="p_eo", bufs=B_EO) as p_eo, \
             tc.tile_pool(name="p_out", bufs=1) as p_out, \
             tc.tile_pool(name="psum", bufs=1, space="PSUM") as p_ps:


            def mm_pair(psrow, lhsT_ap, rhs_of_j, start, stop):
                """Two matmuls sharing one stationary operand: the second
                skips its LDWEIGHTS (identical weights already in the array)
                and is order-pinned right after the first."""
                prev = None
                for j in range(NJ):
                    m = nc.tensor.matmul(psrow[j], lhsT_ap, rhs_of_j(j),
                                         start=start, stop=stop)
                    # fp32's two-pass matmul requires self-loading weights
                    if j > 0 and SKIP_LDW and bf16_mode:
                        m.ins.ldweights = False
                        add_dep_helper(m.ins, prev.ins, False, "weight-reuse pair order")
                    prev = m

            def psum_tiles(tagp):
                return [[p_ps.tile([128, NFREE], dt.float32,
                                   name=f"{tagp}_{i}_{j}", tag=f"ps_{i}_{j}")
                         for j in range(NJ)] for i in range(4)]

            # ------------- Stage 1: eo_e = fT.T @ we, by c-halves ------------
            last_we_dma = None
            for ch in range(2):
                c0 = ch * CH
                psums = psum_tiles(f"s1h{ch}")
                for blk in range(S1BLK):
                    r0 = blk * 128 * S1SUB
                    # only this c-half's columns of fT are needed
                    ft_t = p_ft.tile([128, S1SUB, CH], io_dt, tag="ft", name=f"ft_{ch}_{blk}")
                    we_t = p_we.tile([128, S1SUB, D_OUT], io_dt, tag="we", name=f"we_{ch}_{blk}")
                    if ch == 0 and blk == 0:
                        # per-sub loads: the first matmul starts after 256 KiB,
                        # not after the whole block
                        for sub in range(S1SUB):
                            nc.sync.dma_start(
                                out=ft_t[:, sub:sub + 1, :],
                                in_=rearr(fT[r0 + sub * 128:r0 + (sub + 1) * 128, c0:c0 + CH]))
                            last_we_dma = nc.scalar.dma_start(
                                out=we_t[:, sub:sub + 1, :],
                                in_=rearr(we[r0 + sub * 128:r0 + (sub + 1) * 128, :]))
                    else:
                        nc.sync.dma_start(
                            out=ft_t, in_=rearr(fT[r0:r0 + 128 * S1SUB, c0:c0 + CH]))
                        # scalar queue: second HWDGE ring, parallel with sync's
                        last_we_dma = nc.scalar.dma_start(
                            out=we_t, in_=rearr(we[r0:r0 + 128 * S1SUB, :]))
                    for sub in range(S1SUB):
                        kt = blk * S1SUB + sub
                        for i in range(4):
                            mm_pair(psums[i],
                                    mm_cast(ft_t[:, sub, i * 128:(i + 1) * 128]),
                                    lambda j, sub=sub: mm_cast(we_t[:, sub, j * NFREE:(j + 1) * NFREE]),
                                    start=(kt == 0), stop=(kt == KT - 1))
                eo_half = p_eo.tile([128, 4, D_OUT], io_dt, tag="eo", name=f"eo_{ch}")
                for i in range(4):
                    for j in range(NJ):
                        nc.vector.tensor_copy(out=eo_half[:, i, j * NFREE:(j + 1) * NFREE],
                                              in_=psums[i][j])
                # gpsimd (SWDGE) queue: keeps this late-gated write out of the
                # HWDGE FIFOs so it can't head-of-line block operand loads
                nc.gpsimd.dma_start(out=rearr(ag_in[ch][:]), in_=eo_half)
                # chunked AllGather: starts while the PE grinds the next phase
                nc.gpsimd.collective_compute(
                    "AllGather", mybir.AluOpType.bypass,
                    replica_groups=[list(range(N_CORES))],
                    ins=[ag_in[ch][:].opt()], outs=[ag_out[ch][:].opt()])

            # ------------- Stage 3: resid partial (w1 folded) ----------------
            psums = psum_tiles("s23")
            last_rw_dma = None
            for blk in range(NBLK):
                ri_t = p_ri.tile([128, SUB, S_LOC], io_dt, tag="ri", name=f"ri_{blk}")
                nc.sync.dma_start(out=ri_t, in_=rearr(riT[blk * 512:(blk + 1) * 512, :]))
                rw_t = p_rw.tile([128, SUB, D_OUT], io_dt, tag="rw", name=f"rw_{blk}")
                last_rw_dma = nc.scalar.dma_start(
                    out=rw_t, in_=rearr(rw[blk * 512:(blk + 1) * 512, :]))
                for sub in range(SUB):
                    kt = blk * SUB + sub
                    for i in range(4):
                        mm_pair(psums[i],
                                mm_cast(ri_t[:, sub, i * 128:(i + 1) * 128]),
                                lambda j, sub=sub: mm_cast(rw_t[:, sub, j * NFREE:(j + 1) * NFREE]),
                                start=(kt == 0), stop=False)

            # ------------- Stage 2: combine partial (w0 folded) --------------
            out_sb = p_out.tile([128, 4, D_OUT], dt.float32)
            prev_eoag_dma = None
            for blk in range(ECT // SUB):
                half = blk // 8              # ag chunk this block reads
                r0 = (blk % 8) * 512
                cw_t = p_cw.tile([128, SUB, S_LOC], io_dt, tag="cw", name=f"cw_{blk}")
                nc.sync.dma_start(out=cw_t, in_=rearr(cwT[blk * 512:(blk + 1) * 512, :]))
                eo_t = p_eoag.tile([128, SUB, D_OUT], io_dt, tag="eoag", name=f"eoag_{blk}")
                eoag_dma = nc.scalar.dma_start(out=eo_t, in_=rearr(ag_out[half][r0:r0 + 512, :]))
                # Scalar-queue order: eoag loads wait on the AllGathers, so pin
                # them after every stage-1/3 operand load and in block order —
                # otherwise the scheduler can hoist one ahead and head-of-line
                # block the HWDGE FIFO on the collective (measured 44 us stall).
                prev = prev_eoag_dma if prev_eoag_dma is not None else (
                    last_rw_dma if last_rw_dma is not None else last_we_dma)
                if prev is not None:
                    add_dep_helper(eoag_dma.ins, prev.ins, False,
                                   "eoag after stage-1/3 loads, in block order")
                prev_eoag_dma = eoag_dma
                last_blk = blk == ECT // SUB - 1
                if not last_blk:
                    for sub in range(SUB):
                        for i in range(4):
                            mm_pair(psums[i],
                                    mm_cast(cw_t[:, sub, i * 128:(i + 1) * 128]),
                                    lambda j, sub=sub: mm_cast(eo_t[:, sub, j * NFREE:(j + 1) * NFREE]),
                                    start=False, stop=False)
                else:
                    # last block: finish groups one at a time so the PSUM->SBUF
                    # copies and output DMAs overlap the remaining matmuls
                    for i in range(4):
                        for j in range(NJ):
                            for sub in range(SUB):
                                nc.tensor.matmul(
                                    psums[i][j],
                                    mm_cast(cw_t[:, sub, i * 128:(i + 1) * 128]),
                                    mm_cast(eo_t[:, sub, j * NFREE:(j + 1) * NFREE]),
                                    start=False, stop=(sub == SUB - 1))
                            nc.vector.tensor_copy(
                                out=out_sb[:, i, j * NFREE:(j + 1) * NFREE],
                                in_=psums[i][j])
                        nc.sync.dma_start(
                            out=out[i * 128:(i + 1) * 128, :].rearrange("(n p) d -> p n d", p=128),
                            in_=out_sb[:, i:i + 1, :])

    nc.finalize()
    return nc


def _get_prog(mode, ldw_opt):
    key = (mode, ldw_opt)
    if key not in _prog_cache:
        if ldw_opt:
            _patch_ldw_opt()
        _prog_cache[key] = _build(mode, ldw_opt)
    return _prog_cache[key]


def _prep_in_maps(inputs, expert_w, residual_w, combine_weights, residual_weight, mode):
    np_dt = BF16 if mode == "bf16" else np.float32
    front = inputs[:E * C].reshape(E, C, D_IN)
    resid = inputs[E * C:]                       # [TOK, D_IN]
    rwt = residual_weight.reshape(TOK, 2)
    w0, w1 = rwt[:, 0], rwt[:, 1]

    rw_cast = np.ascontiguousarray(residual_w.astype(np_dt))
    resid_s = resid * w1[:, None]                # fold w1 (fp32)
    in_maps = []
    for r in range(N_CORES):
        sl = slice(r * S_LOC, (r + 1) * S_LOC)
        fT = np.ascontiguousarray(front[r].T.astype(np_dt))              # [D_IN, C]
        we = np.ascontiguousarray(expert_w[r].astype(np_dt))             # [D_IN, D_OUT]
        cw_s = combine_weights[sl] * w0[sl, None, None]                  # [S_LOC, E, C]
        # contraction rows ordered (c-half chunk, expert, c-within-half) to
        # match the chunked AllGather's concatenation
        cwT = np.ascontiguousarray(
            cw_s.reshape(S_LOC, E, 2, CH).transpose(2, 1, 3, 0).reshape(E * C, S_LOC)
            .astype(np_dt))
        riT = np.ascontiguousarray(resid_s[sl].T.astype(np_dt))          # [D_IN, S_LOC]
        in_maps.append({"fT": fT, "we": we, "cwT": cwT, "riT": riT, "rw": rw_cast})
    return in_maps


def _run(inputs, expert_w, expert_b, residual_w, residual_b,
         combine_weights, residual_weight, mode=None, ldw_opt=None, trace=False):
    import jax
    try:
        if jax.config.jax_compilation_cache_dir is None:
            jax.config.update("jax_compilation_cache_dir", "/tmp/jax_cache_trn_moe")
            jax.config.update("jax_persistent_cache_min_compile_time_secs", 0.5)
    except Exception:
        pass
    from concourse.bass_utils import run_bass_kernel_spmd

    mode = mode or MODE
    ldw_opt = LDW_OPT if ldw_opt is None else ldw_opt
    inputs = np.asarray(inputs, dtype=np.float32)
    expert_w = np.asarray(expert_w, dtype=np.float32)
    expert_b = np.asarray(expert_b, dtype=np.float32)
    residual_w = np.asarray(residual_w, dtype=np.float32)
    residual_b = np.asarray(residual_b, dtype=np.float32)
    combine_weights = np.asarray(combine_weights, dtype=np.float32)
    residual_weight = np.asarray(residual_weight, dtype=np.float32)

    nc = _get_prog(mode, ldw_opt)
    in_maps = _prep_in_maps(inputs, expert_w, residual_w, combine_weights,
                            residual_weight, mode)
    res = run_bass_kernel_spmd(nc, in_maps, list(range(N_CORES)), trace=trace)
    out = np.concatenate([res.results[r]["out"] for r in range(N_CORES)], axis=0)

    # exact bias contributions (zero in practice, but keep the math honest)
    rwt = residual_weight.reshape(TOK, 2)
    if residual_b.any():
        out = out + rwt[:, 1:2] * residual_b[None, :]
    if expert_b.any():
        cs = combine_weights.sum(axis=2)                    # [TOK, E]
        out = out + rwt[:, 0:1] * (cs @ expert_b)
    return out.reshape(B, S, D_OUT).astype(np.float32), res


def kernel(**kw):
    out, _ = _run(**kw)
    return out
